# revision 44
# baseline (speedup 1.0000x reference)
"""Trainium2 Bass kernel for nn_NeuralALU (batched byte-encoded 32-bit add).

The reference network is a chain of table-lookup matmuls + sharp softmaxes
(scale=100) over exactly-one-hot byte encodings. Because the inputs are exact
one-hots, the float pipeline collapses to a discrete algorithm (validated to
float-exactness against the jax reference in validate_math.py):

  z[2j]   = a_j%16 + b_j%16        (lo nibble sum of byte j, 0..30)
  z[2j+1] = a_j//16 + b_j//16      (hi nibble sum)
  carry c in {0, 0.5, 1}, init 0.5, chained over nibbles lo0,hi0,...,hi3:
      c' = max(g, min(p, c)),  g = [x>=16], p = [x>=15]
  add = [c==1]; y = x+add; U = y mod 16; Pf = [c==0.5]
  nibble dist = onehot(U)*(1-Pf/2) + onehot(U+1 mod 16)*(Pf/2)
  out byte row [256] = outer(hi_dist, lo_dist)

All staged values are exactly representable in the staging dtypes: the 0/1
one-hots and small-int nibble tables in fp8 e4m3, intermediate sums (<=31),
dist weights {0,.5,1} and outputs {0,.25,.5,1} in bf16 — so the kernel is
value-identical to the fp32 computation at a fraction of the HBM traffic.

Layout/engine strategy (pure data parallel over batch, 4096 rows/core):
 - Host pre-transposes each core's inputs to abT [2048 feat, 4096 rows] fp8
   so the (otherwise idle) tensor engine does the whole one-hot->nibble-sum
   extraction as matmuls against tiny [128, 2] nibble-value tables, packed
   4-wide into PE column groups {0,32,64,96} via tile_position (byte j's four
   K-chunks accumulate in its own column group -> no cross-group reduction).
 - ScalarE (also otherwise idle) drains PSUM; TensorE transposes z back to
   rows-on-partitions; the carry chain is ONE tensor_tensor_scan per chunk;
   dist build on DVE; the 256-wide outer products are split DVE/GPSIMD in
   4-row-tile quads; outputs stream back as bf16 in 1 MiB DMAs on the ACT
   HWDGE ring (inputs use SP's).
"""

import numpy as np
import ml_dtypes

import concourse.bass as bass
import concourse.bacc as bacc
import concourse.mybir as mybir
from concourse.tile import TileContext
from concourse.bass_utils import run_bass_kernel_spmd

N_CORES = 8
B_FULL = 32768
ROWS = B_FULL // N_CORES  # 4096 rows per core
F = 1024                  # 4 bytes x 256 one-hot
P = 128
NTC = 8                   # row-tiles per chunk (1024 rows = one input group)
HG = 512                  # matmul moving free dim (rows per half-group)

FP = mybir.dt.float32
BF = mybir.dt.bfloat16
F8 = mybir.dt.float8e4

# per (chunk, quad): how many of the quad's 4 row-tiles run their outer
# products on DVE (the rest go to GPSIMD); tuned from occupancy profiles
DVE_TILES = {(0, 0): 2, (0, 1): 2, (1, 0): 2, (1, 1): 2,
             (2, 0): 2, (2, 1): 2, (3, 0): 2, (3, 1): 4}
# lrep/hrep staging engine alternates ScalarE/GPSIMD to balance their load
REP_ON_GPSIMD_FRAC = 3  # every 3rd rep-copy pair goes to GPSIMD


def _host_tables():
    # K-chunk c of abT covers features [128c, 128c+128) = half of one byte's
    # 256-wide one-hot block; its code k = 128*(c%2) + p. The per-chunk
    # [128, 2] nibble-value table (col 0: k%16, col 1: k//16) therefore only
    # depends on c's parity: tab[:, 2*(c%2) : 2*(c%2)+2].
    p = np.arange(128)
    tab = np.zeros((128, 4), np.float32)
    tab[:, 0] = p % 16
    tab[:, 1] = p // 16
    tab[:, 2] = p % 16
    tab[:, 3] = 8 + p // 16
    ident = np.eye(P, dtype=np.float32)
    # iota17[k] = (k+15) % 16 so one compare row yields onehot(U) at cols
    # 1..16 and onehot((U+1)%16) at cols 0..15
    i17 = ((np.arange(17) + 15) % 16).astype(np.float32)
    iota17 = np.broadcast_to(i17, (P, 17))
    return (
        np.ascontiguousarray(tab).astype(ml_dtypes.float8_e4m3),
        np.ascontiguousarray(ident),
        np.ascontiguousarray(iota17).astype(ml_dtypes.bfloat16),
    )


def build_nc(rows=ROWS):
    AL = mybir.AluOpType
    n_chunks = rows // (P * NTC)

    nc = bacc.Bacc()
    abT_d = nc.declare_dram_parameter("abT", [2 * F, rows], F8, isOutput=False)
    tab_d = nc.declare_dram_parameter("tab", [P, 4], F8, isOutput=False)
    ident_d = nc.declare_dram_parameter("ident", [P, P], FP, isOutput=False)
    iota_d = nc.declare_dram_parameter("iota17", [P, 17], BF, isOutput=False)
    out_d = nc.declare_dram_parameter("out", [rows, F], BF, isOutput=True)

    # input view: chunk c (of 16), partition p, row r  ->  abT[(c,p), r]
    ab_v = abT_d[:, :].rearrange("(c p) r -> p c r", p=P)
    # quad output view: quad u -> [p, t4, f] (4 row-tiles per DMA)
    out4_v = out_d[:, :].rearrange("(u t4 p) f -> u p t4 f", t4=4, p=P)

    with TileContext(nc) as tc:
        with (
            tc.tile_pool(name="consts", bufs=1) as cpool,
            tc.tile_pool(name="io", bufs=2) as iopool,
            tc.tile_pool(name="zsb", bufs=4) as zpool,
            tc.tile_pool(name="arrs", bufs=2) as apool,
            tc.tile_pool(name="dist", bufs=2) as dpool,
            tc.tile_pool(name="outp", bufs=3) as opool,
            tc.tile_pool(name="lrep", bufs=4) as lpool,
            tc.tile_pool(name="psz", bufs=2, space="PSUM") as psumz,
            tc.tile_pool(name="pst", bufs=4, space="PSUM") as psumt,
        ):
            tab_sb = cpool.tile([P, 4], F8, tag="tab")
            ident_sb = cpool.tile([P, P], FP, tag="ident")
            iota17 = cpool.tile([P, 17], BF, tag="iota17")
            nc.sync.dma_start(tab_sb[:, :], tab_d[:, :])
            nc.sync.dma_start(ident_sb[:, :], ident_d[:, :])
            nc.sync.dma_start(iota17[:, :], iota_d[:, :])

            # transposes lag the matmul stream by one half-group so the
            # PSUM-drain (zcopy) wait never stalls the PE's matmul queue
            pending_tr = []  # (zsb, xnib, chunk-local row-tile base)

            def flush_one_transpose_batch():
                if not pending_tr:
                    return
                zsb, xnib, tl0, ntiles = pending_tr.pop(0)
                for tt in range(ntiles):
                    tl = tl0 + tt
                    pst = psumt.tile([P, P], FP, tag="pst")
                    nc.tensor.transpose(
                        pst[:, :], zsb[:, tt * P : (tt + 1) * P], ident_sb[:, :]
                    )
                    # z columns live at partitions {32j, 32j+1} per byte j
                    # -> strided view recovers carry order lo0,hi0,...
                    pz_v = pst[:, :].rearrange("p (g q) -> p g q", g=4)[:, :, 0:2]
                    nc.scalar.copy(
                        xnib[:, tl * 8 : (tl + 1) * 8].rearrange(
                            "p (g q) -> p g q", g=4
                        ),
                        pz_v,
                    )

            def extract(ch):
                """Input DMA + matmul extraction; returns the xnib tile."""
                xnib = apool.tile([P, NTC * 8], BF, tag="xnib")
                grp = iopool.tile([P, 16 * 2 * HG], F8, tag="grp")
                grp_v = grp[:, :].rearrange("p (c r) -> p c r", c=16)
                r0 = ch * 2 * HG
                # chunk 0 split per half-group to shorten startup
                if ch == 0:
                    for hh in range(2):
                        nc.sync.dma_start(
                            grp_v[:, :, hh * HG : (hh + 1) * HG],
                            ab_v[:, :, r0 + hh * HG : r0 + (hh + 1) * HG],
                        )
                else:
                    nc.sync.dma_start(grp_v, ab_v[:, :, r0 : r0 + 2 * HG])

                # matmuls packed 4-wide into PE column groups {0,32,64,96}
                # (byte j's four K-chunks accumulate in its own column
                # group). N=512 per matmul (the ISA rejects N=1024:
                # s3d3_mm_num_elements).
                n_mm = 2
                W = (2 * HG) // n_mm
                for hh in range(n_mm):
                    psz = psumz.tile([P, 2 * HG], FP, tag="psz")
                    for seq in range(4):
                        for j in range(4):
                            # chunks of byte j: a: 2j, 2j+1; b: 8+2j, 8+2j+1
                            c = (seq % 2) + 2 * j + 8 * (seq // 2)
                            nc.tensor.matmul(
                                psz[32 * j : 32 * j + 2, 0:W],
                                tab_sb[:, 2 * (c % 2) : 2 * (c % 2) + 2],
                                grp_v[:, c, hh * W : (hh + 1) * W],
                                start=(seq == 0),
                                stop=(seq == 3),
                                skip_group_check=True,
                                tile_position=(0, 32 * j),
                            )
                    zsb = zpool.tile([P, 2 * HG], FP, tag="zsb")
                    nc.scalar.copy(zsb[:, 0:W], psz[:, 0:W])
                    flush_one_transpose_batch()
                    pending_tr.append((zsb, xnib, hh * (8 // n_mm), 8 // n_mm))
                return xnib

            def process(ch, xnib):
                # ---- carry scan over [reset-padded] nibble chains ----
                gp_g = apool.tile([P, NTC * 9], BF, tag="gpg")
                gp_p = apool.tile([P, NTC * 9], BF, tag="gpp")
                cbuf = apool.tile([P, NTC * 9 + 1], BF, tag="cbuf")
                gg_v = gp_g[:, :].rearrange("p (t n) -> p t n", n=9)
                pp_v = gp_p[:, :].rearrange("p (t n) -> p t n", n=9)
                xn_v = xnib[:, :].rearrange("p (t n) -> p t n", n=8)
                nc.vector.tensor_scalar(
                    out=gg_v[:, :, 0:8], in0=xn_v, scalar1=15.5, scalar2=None,
                    op0=AL.is_ge,
                )
                nc.vector.tensor_scalar(
                    out=pp_v[:, :, 0:8], in0=xn_v, scalar1=14.5, scalar2=None,
                    op0=AL.is_ge,
                )
                nc.vector.memset(gg_v[:, :, 8:9], 0.5)
                nc.vector.memset(pp_v[:, :, 8:9], 0.5)
                nc.vector.memset(cbuf[:, 0:1], 0.5)
                nc.vector.tensor_tensor_scan(
                    out=cbuf[:, 1 : NTC * 9 + 1],
                    data0=gp_p[:, :],
                    data1=gp_g[:, :],
                    initial=0.5,
                    op0=AL.min,
                    op1=AL.max,
                )
                c_pre = cbuf[:, 0 : NTC * 9].rearrange("p (t n) -> p t n", n=9)[
                    :, :, 0:8
                ]

                # ---- U / P-flag / dist weights ----
                add_a = apool.tile([P, NTC * 8], BF, tag="adda")
                pf = apool.tile([P, NTC * 8], BF, tag="pf")
                y_a = apool.tile([P, NTC * 8], BF, tag="ya")
                wrap = apool.tile([P, NTC * 8], BF, tag="wrap")
                u_a = apool.tile([P, NTC * 8], BF, tag="ua")
                w0 = apool.tile([P, NTC * 8], BF, tag="w0")
                w1 = apool.tile([P, NTC * 8], BF, tag="w1")
                av = add_a[:, :].rearrange("p (t n) -> p t n", n=8)
                pv = pf[:, :].rearrange("p (t n) -> p t n", n=8)
                nc.vector.tensor_scalar(
                    out=av, in0=c_pre, scalar1=0.75, scalar2=None, op0=AL.is_ge
                )
                nc.vector.tensor_scalar(
                    out=pv, in0=c_pre, scalar1=0.5, scalar2=None, op0=AL.is_equal
                )
                nc.vector.tensor_add(y_a[:, :], xnib[:, :], add_a[:, :])
                nc.vector.tensor_scalar(
                    out=wrap[:, :], in0=y_a[:, :], scalar1=15.5, scalar2=None,
                    op0=AL.is_ge,
                )
                nc.vector.scalar_tensor_tensor(
                    out=u_a[:, :], in0=wrap[:, :], scalar=-16.0, in1=y_a[:, :],
                    op0=AL.mult, op1=AL.add,
                )
                # w0/w1 are pure scale/bias of pf -> offload to idle ScalarE
                nc.scalar.mul(w1[:, :], pf[:, :], 0.5)
                nc.scalar.activation(
                    w0[:, :], pf[:, :], mybir.ActivationFunctionType.Identity,
                    bias=1.0, scale=-0.5,
                )

                # ---- dist build: one iota-compare then weighted combine
                #      (GPSIMD rejects compare ops — Pool engine ISA) ----
                TN = NTC * 8
                eqx = dpool.tile([P, TN * 17], BF, tag="eqx")
                dsub = dpool.tile([P, TN * 16], BF, tag="dsub")
                dtmp = dpool.tile([P, TN * 16], BF, tag="dtmp")
                # eqx[tn, k] = [U[tn] == (k+15)%16]: cols 1..16 = onehot(U),
                # cols 0..15 = onehot((U+1)%16)
                eq_v = eqx[:, :].rearrange("p (tn k) -> p tn k", k=17)
                u_b = u_a[:, :, None].broadcast_to([P, TN, 17])
                io_b = iota17[:, None, :].broadcast_to([P, TN, 17])
                nc.vector.tensor_tensor(eq_v, u_b, io_b, op=AL.is_equal)
                ds_v = dsub[:, :].rearrange("p (tn k) -> p tn k", k=16)
                dt_v = dtmp[:, :].rearrange("p (tn k) -> p tn k", k=16)
                w0_b = w0[:, :, None].broadcast_to([P, TN, 16])
                w1_b = w1[:, :, None].broadcast_to([P, TN, 16])
                nc.vector.tensor_mul(ds_v, eq_v[:, :, 1:17], w0_b)
                nc.vector.tensor_mul(dt_v, eq_v[:, :, 0:16], w1_b)
                nc.vector.tensor_add(dsub[:, :], dsub[:, :], dtmp[:, :])

                # merged contiguous copy of the hi dists -> the DVE outer's
                # h operand becomes a 3D AP (ScalarE pays the strided read)
                dhm = dpool.tile([P, NTC * 64], BF, tag="dhm")
                dv = dsub[:, :].rearrange(
                    "p (t i hf k) -> p t i hf k", i=4, hf=2, k=16
                )
                nc.scalar.copy(
                    dhm[:, :].rearrange("p (t i k) -> p t i k", i=4, k=16),
                    dv[:, :, :, 1, :],
                )
                dhm_v = dhm[:, :].rearrange("p (t ihk) -> p t ihk", t=NTC)

                # ---- outer products in 4-row-tile quads, split DVE/GPSIMD.
                #      DVE tiles get a ScalarE-staged contiguous l_rep so the
                #      TT's operands have <=2 free dims (the 3-free-dim
                #      broadcast AP runs ~2.4x slower on DVE) ----
                for q in range(NTC // 4):
                    u_idx = ch * (NTC // 4) + q
                    n_dve = DVE_TILES[(ch, q)]
                    o4 = opool.tile([P, 4 * F], BF, tag="o4")
                    for t4 in range(4):
                        o_v = o4[:, t4 * F : (t4 + 1) * F].rearrange(
                            "p (i h k) -> p i h k", h=16, k=16
                        )
                        tl = q * 4 + t4
                        if t4 >= 4 - n_dve:
                            # materialize BOTH operands contiguously (a
                            # broadcast operand in the DVE TT trips a
                            # 2.4x-slower perf-mode path, measured; two
                            # clean contiguous bf16 operands run true
                            # 2x_1P). Staging copies alternate between the
                            # otherwise-idle ScalarE and GPSIMD.
                            on_gps = (
                                (ch * 8 + q * 4 + t4) % REP_ON_GPSIMD_FRAC
                                == 1
                            )

                            def stage(dst, src):
                                if on_gps:
                                    nc.gpsimd.tensor_copy(dst, src)
                                else:
                                    nc.scalar.copy(dst, src)

                            lrep = lpool.tile([P, F], BF, tag="lrep")
                            stage(
                                lrep[:, :].rearrange(
                                    "p (i h k) -> p i h k", h=16, k=16
                                ),
                                dv[:, tl, :, 0, :][:, :, None, :].broadcast_to(
                                    [P, 4, 16, 16]
                                ),
                            )
                            hrep = lpool.tile([P, F], BF, tag="hrep")
                            stage(
                                hrep[:, :].rearrange(
                                    "p (a k) -> p a k", k=16
                                ),
                                dhm_v[:, tl, :, None].broadcast_to(
                                    [P, 64, 16]
                                ),
                            )
                            nc.vector.tensor_mul(
                                o4[:, t4 * F : (t4 + 1) * F],
                                lrep[:, :],
                                hrep[:, :],
                            )
                        else:
                            l_b = dv[:, tl, :, 0, :][:, :, None, :].broadcast_to(
                                [P, 4, 16, 16]
                            )
                            h_b = dv[:, tl, :, 1, :][:, :, :, None].broadcast_to(
                                [P, 4, 16, 16]
                            )
                            nc.gpsimd.tensor_mul(o_v, l_b, h_b)
                    # early outputs ride the ACT HWDGE ring (SP is busy with
                    # the input stream); later ones use the by-then-idle SP
                    dma_eng = nc.scalar if ch == 0 else nc.sync
                    dma_eng.dma_start(out4_v[u_idx], o4[:, :])

            # ---- software pipeline: process(ch) runs while chunk ch+1's
            #      matmuls stream on the PE ----
            prev = None
            for ch in range(n_chunks):
                xnib = extract(ch)
                if prev is not None:
                    process(*prev)
                prev = (ch, xnib)
            flush_one_transpose_batch()
            flush_one_transpose_batch()
            process(*prev)

    nc.finalize()
    return nc


_NC_CACHE = {}
LAST_RESULT = None


def kernel(**inputs) -> np.ndarray:
    global LAST_RESULT
    a = np.ascontiguousarray(np.asarray(inputs["a"], dtype=np.float32)).reshape(
        B_FULL, F
    )
    b = np.ascontiguousarray(np.asarray(inputs["b"], dtype=np.float32)).reshape(
        B_FULL, F
    )
    # fp8 e4m3 staging: the one-hots are exactly 0.0/1.0 -> bytes 0x00/0x38.
    a8 = (a.view(np.uint16)[:, 1::2] != 0).astype(np.uint8) * np.uint8(0x38)
    b8 = (b.view(np.uint16)[:, 1::2] != 0).astype(np.uint8) * np.uint8(0x38)
    aT = np.ascontiguousarray(a8.reshape(N_CORES, ROWS, F).transpose(0, 2, 1))
    bT = np.ascontiguousarray(b8.reshape(N_CORES, ROWS, F).transpose(0, 2, 1))
    tab, ident, iota17 = _host_tables()

    if ROWS not in _NC_CACHE:
        _NC_CACHE[ROWS] = build_nc(ROWS)
    nc = _NC_CACHE[ROWS]

    in_maps = []
    for c in range(N_CORES):
        abT = np.concatenate([aT[c], bT[c]], axis=0).view(ml_dtypes.float8_e4m3)
        in_maps.append({"abT": abT, "tab": tab, "ident": ident, "iota17": iota17})
    res = run_bass_kernel_spmd(nc, in_maps, core_ids=list(range(N_CORES)))
    LAST_RESULT = res
    out16 = np.concatenate([r["out"] for r in res.results], axis=0)
    # bf16 -> fp32 exact expansion
    out32 = (out16.view(np.uint16).astype(np.uint32) << 16).view(np.float32)
    return out32.reshape(B_FULL, 4, 256)


# revision 49
# speedup vs baseline: 1.4924x; 1.4924x over previous
"""Trainium2 Bass kernel for nn_NeuralALU (batched byte-encoded 32-bit add).

The reference network is a chain of table-lookup matmuls + sharp softmaxes
(scale=100) over exactly-one-hot byte encodings. Because the inputs are exact
one-hots, the float pipeline collapses to a discrete algorithm (validated to
float-exactness against the jax reference in validate_math.py):

  z[2j]   = a_j%16 + b_j%16        (lo nibble sum of byte j, 0..30)
  z[2j+1] = a_j//16 + b_j//16      (hi nibble sum)
  carry c in {0, 0.5, 1}, init 0.5, chained over nibbles lo0,hi0,...,hi3:
      c' = max(g, min(p, c)),  g = [x>=16], p = [x>=15]
  add = [c==1]; y = x+add; U = y mod 16; Pf = [c==0.5]
  nibble dist = onehot(U)*(1-Pf/2) + onehot(U+1 mod 16)*(Pf/2)
  out byte row [256] = outer(hi_dist, lo_dist)

All staged values are exactly representable in the staging dtypes: the 0/1
one-hots and small-int nibble tables in fp8 e4m3, intermediate sums (<=31),
dist weights {0,.5,1} and outputs {0,.25,.5,1} in bf16 — so the kernel is
value-identical to the fp32 computation at a fraction of the HBM traffic.

Layout/engine strategy (pure data parallel over batch, 4096 rows/core):
 - Host pre-transposes each core's inputs to abT [2048 feat, 4096 rows] fp8
   so the (otherwise idle) tensor engine does the whole one-hot->nibble-sum
   extraction as matmuls against tiny [128, 2] nibble-value tables, packed
   4-wide into PE column groups {0,32,64,96} via tile_position (byte j's four
   K-chunks accumulate in its own column group -> no cross-group reduction).
 - ScalarE (also otherwise idle) drains PSUM; TensorE transposes z back to
   rows-on-partitions; the carry chain is ONE tensor_tensor_scan per chunk;
   dist build on DVE; the 256-wide outer products are split DVE/GPSIMD in
   4-row-tile quads; outputs stream back as bf16 in 1 MiB DMAs on the ACT
   HWDGE ring (inputs use SP's).
"""

import numpy as np
import ml_dtypes

import concourse.bass as bass
import concourse.bacc as bacc
import concourse.mybir as mybir
from concourse.tile import TileContext
from concourse.bass_utils import run_bass_kernel_spmd

N_CORES = 8
B_FULL = 32768
ROWS = B_FULL // N_CORES  # 4096 rows per core
F = 1024                  # 4 bytes x 256 one-hot
P = 128
NTC = 8                   # row-tiles per chunk (1024 rows = one input group)
HG = 512                  # matmul moving free dim (rows per half-group)

FP = mybir.dt.float32
BF = mybir.dt.bfloat16
F8 = mybir.dt.float8e4

# per (chunk, quad): how many of the quad's 4 row-tiles run their outer
# products on DVE (the rest go to GPSIMD); tuned from occupancy profiles
DVE_TILES = {(0, 0): 2, (0, 1): 2, (1, 0): 2, (1, 1): 2,
             (2, 0): 2, (2, 1): 2, (3, 0): 2, (3, 1): 4}
# broadcast-source staging copies run ONLY on ScalarE — GPSIMD executes
# them at ~4.4us/tile (5c/elem on stride-0 reads), measured


def _host_tables():
    # K-chunk c of abT covers features [128c, 128c+128) = half of one byte's
    # 256-wide one-hot block; its code k = 128*(c%2) + p. The per-chunk
    # [128, 2] nibble-value table (col 0: k%16, col 1: k//16) therefore only
    # depends on c's parity: tab[:, 2*(c%2) : 2*(c%2)+2].
    p = np.arange(128)
    tab = np.zeros((128, 4), np.float32)
    tab[:, 0] = p % 16
    tab[:, 1] = p // 16
    tab[:, 2] = p % 16
    tab[:, 3] = 8 + p // 16
    ident = np.eye(P, dtype=np.float32)
    # iota17[k] = (k+15) % 16 so one compare row yields onehot(U) at cols
    # 1..16 and onehot((U+1)%16) at cols 0..15
    i17 = ((np.arange(17) + 15) % 16).astype(np.float32)
    iota17 = np.broadcast_to(i17, (P, 17))
    return (
        np.ascontiguousarray(tab).astype(ml_dtypes.float8_e4m3),
        np.ascontiguousarray(ident),
        np.ascontiguousarray(iota17).astype(ml_dtypes.bfloat16),
    )


def build_nc(rows=ROWS):
    AL = mybir.AluOpType
    n_chunks = rows // (P * NTC)

    nc = bacc.Bacc()
    abT_d = nc.declare_dram_parameter("abT", [2 * F, rows], F8, isOutput=False)
    tab_d = nc.declare_dram_parameter("tab", [P, 4], F8, isOutput=False)
    ident_d = nc.declare_dram_parameter("ident", [P, P], FP, isOutput=False)
    iota_d = nc.declare_dram_parameter("iota17", [P, 17], BF, isOutput=False)
    out_d = nc.declare_dram_parameter("out", [rows, F], BF, isOutput=True)

    # input view: chunk c (of 16), partition p, row r  ->  abT[(c,p), r]
    ab_v = abT_d[:, :].rearrange("(c p) r -> p c r", p=P)
    # quad output view: quad u -> [p, t4, f] (4 row-tiles per DMA)
    out4_v = out_d[:, :].rearrange("(u t4 p) f -> u p t4 f", t4=4, p=P)

    with TileContext(nc) as tc:
        with (
            tc.tile_pool(name="consts", bufs=1) as cpool,
            tc.tile_pool(name="io", bufs=2) as iopool,
            tc.tile_pool(name="zsb", bufs=4) as zpool,
            tc.tile_pool(name="arrs", bufs=2) as apool,
            tc.tile_pool(name="dist", bufs=2) as dpool,
            tc.tile_pool(name="outp", bufs=3) as opool,
            tc.tile_pool(name="lrep", bufs=4) as lpool,
            tc.tile_pool(name="psz", bufs=2, space="PSUM") as psumz,
            tc.tile_pool(name="pst", bufs=5, space="PSUM") as psumt,
        ):
            tab_sb = cpool.tile([P, 4], F8, tag="tab")
            ident_sb = cpool.tile([P, P], FP, tag="ident")
            iota17 = cpool.tile([P, 17], BF, tag="iota17")
            nc.sync.dma_start(tab_sb[:, :], tab_d[:, :])
            nc.sync.dma_start(ident_sb[:, :], ident_d[:, :])
            nc.sync.dma_start(iota17[:, :], iota_d[:, :])

            # transposes lag the matmul stream by one half-group so the
            # PSUM-drain (zcopy) wait never stalls the PE's matmul queue
            pending_tr = []  # (zsb, xnib, chunk-local row-tile base)

            def flush_one_transpose_batch():
                if not pending_tr:
                    return
                zsb, xnib, tl0, ntiles = pending_tr.pop(0)
                for tt in range(ntiles):
                    tl = tl0 + tt
                    pst = psumt.tile([P, P], FP, tag="pst")
                    nc.tensor.transpose(
                        pst[:, :], zsb[:, tt * P : (tt + 1) * P], ident_sb[:, :]
                    )
                    # z columns live at partitions {32j, 32j+1} per byte j
                    # -> strided view recovers carry order lo0,hi0,...
                    pz_v = pst[:, :].rearrange("p (g q) -> p g q", g=4)[:, :, 0:2]
                    nc.scalar.copy(
                        xnib[:, tl * 8 : (tl + 1) * 8].rearrange(
                            "p (g q) -> p g q", g=4
                        ),
                        pz_v,
                    )

            def extract(ch):
                """Input DMA + matmul extraction; returns the xnib tile."""
                xnib = apool.tile([P, NTC * 8], BF, tag="xnib")
                grp = iopool.tile([P, 16 * 2 * HG], F8, tag="grp")
                grp_v = grp[:, :].rearrange("p (c r) -> p c r", c=16)
                r0 = ch * 2 * HG
                # chunk 0 split per half-group to shorten startup
                if ch == 0:
                    for hh in range(2):
                        nc.sync.dma_start(
                            grp_v[:, :, hh * HG : (hh + 1) * HG],
                            ab_v[:, :, r0 + hh * HG : r0 + (hh + 1) * HG],
                        )
                else:
                    nc.sync.dma_start(grp_v, ab_v[:, :, r0 : r0 + 2 * HG])

                # matmuls packed 4-wide into PE column groups {0,32,64,96}
                # (byte j's four K-chunks accumulate in its own column
                # group). N=512 per matmul (the ISA rejects N=1024:
                # s3d3_mm_num_elements).
                n_mm = 2
                W = (2 * HG) // n_mm
                for hh in range(n_mm):
                    psz = psumz.tile([P, W], FP, tag="psz")
                    for seq in range(4):
                        for j in range(4):
                            # chunks of byte j: a: 2j, 2j+1; b: 8+2j, 8+2j+1
                            c = (seq % 2) + 2 * j + 8 * (seq // 2)
                            nc.tensor.matmul(
                                psz[32 * j : 32 * j + 2, 0:W],
                                tab_sb[:, 2 * (c % 2) : 2 * (c % 2) + 2],
                                grp_v[:, c, hh * W : (hh + 1) * W],
                                start=(seq == 0),
                                stop=(seq == 3),
                                skip_group_check=True,
                                tile_position=(0, 32 * j),
                            )
                    zsb = zpool.tile([P, W], FP, tag="zsb")
                    nc.scalar.copy(zsb[:, 0:W], psz[:, 0:W])
                    flush_one_transpose_batch()
                    pending_tr.append((zsb, xnib, hh * (8 // n_mm), 8 // n_mm))
                return xnib

            def process(ch, xnib):
                # ---- carry scan over [reset-padded] nibble chains ----
                gp_g = apool.tile([P, NTC * 9], BF, tag="gpg")
                gp_p = apool.tile([P, NTC * 9], BF, tag="gpp")
                cbuf = apool.tile([P, NTC * 9 + 1], BF, tag="cbuf")
                gg_v = gp_g[:, :].rearrange("p (t n) -> p t n", n=9)
                pp_v = gp_p[:, :].rearrange("p (t n) -> p t n", n=9)
                xn_v = xnib[:, :].rearrange("p (t n) -> p t n", n=8)
                nc.vector.tensor_scalar(
                    out=gg_v[:, :, 0:8], in0=xn_v, scalar1=15.5, scalar2=None,
                    op0=AL.is_ge,
                )
                nc.vector.tensor_scalar(
                    out=pp_v[:, :, 0:8], in0=xn_v, scalar1=14.5, scalar2=None,
                    op0=AL.is_ge,
                )
                nc.vector.memset(gg_v[:, :, 8:9], 0.5)
                nc.vector.memset(pp_v[:, :, 8:9], 0.5)
                nc.vector.memset(cbuf[:, 0:1], 0.5)
                nc.vector.tensor_tensor_scan(
                    out=cbuf[:, 1 : NTC * 9 + 1],
                    data0=gp_p[:, :],
                    data1=gp_g[:, :],
                    initial=0.5,
                    op0=AL.min,
                    op1=AL.max,
                )
                c_pre = cbuf[:, 0 : NTC * 9].rearrange("p (t n) -> p t n", n=9)[
                    :, :, 0:8
                ]

                # ---- U / P-flag / dist weights ----
                add_a = apool.tile([P, NTC * 8], BF, tag="adda")
                pf = apool.tile([P, NTC * 8], BF, tag="pf")
                y_a = apool.tile([P, NTC * 8], BF, tag="ya")
                wrap = apool.tile([P, NTC * 8], BF, tag="wrap")
                u_a = apool.tile([P, NTC * 8], BF, tag="ua")
                w0 = apool.tile([P, NTC * 8], BF, tag="w0")
                w1 = apool.tile([P, NTC * 8], BF, tag="w1")
                av = add_a[:, :].rearrange("p (t n) -> p t n", n=8)
                pv = pf[:, :].rearrange("p (t n) -> p t n", n=8)
                nc.vector.tensor_scalar(
                    out=av, in0=c_pre, scalar1=0.75, scalar2=None, op0=AL.is_ge
                )
                nc.vector.tensor_scalar(
                    out=pv, in0=c_pre, scalar1=0.5, scalar2=None, op0=AL.is_equal
                )
                nc.vector.tensor_add(y_a[:, :], xnib[:, :], add_a[:, :])
                nc.vector.tensor_scalar(
                    out=wrap[:, :], in0=y_a[:, :], scalar1=15.5, scalar2=None,
                    op0=AL.is_ge,
                )
                nc.vector.scalar_tensor_tensor(
                    out=u_a[:, :], in0=wrap[:, :], scalar=-16.0, in1=y_a[:, :],
                    op0=AL.mult, op1=AL.add,
                )
                # w0/w1 are pure scale/bias of pf -> offload to idle ScalarE
                nc.scalar.mul(w1[:, :], pf[:, :], 0.5)
                nc.scalar.activation(
                    w0[:, :], pf[:, :], mybir.ActivationFunctionType.Identity,
                    bias=1.0, scale=-0.5,
                )

                # ---- dist build: one iota-compare then weighted combine
                #      (GPSIMD rejects compare ops — Pool engine ISA) ----
                TN = NTC * 8
                eqx = dpool.tile([P, TN * 17], BF, tag="eqx")
                dsub = dpool.tile([P, TN * 16], BF, tag="dsub")
                dtmp = dpool.tile([P, TN * 16], BF, tag="dtmp")
                # eqx[tn, k] = [U[tn] == (k+15)%16]: cols 1..16 = onehot(U),
                # cols 0..15 = onehot((U+1)%16)
                eq_v = eqx[:, :].rearrange("p (tn k) -> p tn k", k=17)
                u_b = u_a[:, :, None].broadcast_to([P, TN, 17])
                io_b = iota17[:, None, :].broadcast_to([P, TN, 17])
                nc.vector.tensor_tensor(eq_v, u_b, io_b, op=AL.is_equal)
                ds_v = dsub[:, :].rearrange("p (tn k) -> p tn k", k=16)
                dt_v = dtmp[:, :].rearrange("p (tn k) -> p tn k", k=16)
                w0_b = w0[:, :, None].broadcast_to([P, TN, 16])
                w1_b = w1[:, :, None].broadcast_to([P, TN, 16])
                nc.vector.tensor_mul(ds_v, eq_v[:, :, 1:17], w0_b)
                nc.vector.tensor_mul(dt_v, eq_v[:, :, 0:16], w1_b)
                nc.vector.tensor_add(dsub[:, :], dsub[:, :], dtmp[:, :])

                # merged contiguous copy of the hi dists -> the DVE outer's
                # h operand becomes a 3D AP (ScalarE pays the strided read)
                dhm = dpool.tile([P, NTC * 64], BF, tag="dhm")
                dv = dsub[:, :].rearrange(
                    "p (t i hf k) -> p t i hf k", i=4, hf=2, k=16
                )
                nc.scalar.copy(
                    dhm[:, :].rearrange("p (t i k) -> p t i k", i=4, k=16),
                    dv[:, :, :, 1, :],
                )
                dhm_v = dhm[:, :].rearrange("p (t ihk) -> p t ihk", t=NTC)

                # ---- outer products in 4-row-tile quads, split DVE/GPSIMD.
                #      DVE tiles get a ScalarE-staged contiguous l_rep so the
                #      TT's operands have <=2 free dims (the 3-free-dim
                #      broadcast AP runs ~2.4x slower on DVE) ----
                for q in range(NTC // 4):
                    u_idx = ch * (NTC // 4) + q
                    n_dve = DVE_TILES[(ch, q)]
                    o4 = opool.tile([P, 4 * F], BF, tag="o4")
                    for t4 in range(4):
                        o_v = o4[:, t4 * F : (t4 + 1) * F].rearrange(
                            "p (i h k) -> p i h k", h=16, k=16
                        )
                        tl = q * 4 + t4
                        if t4 >= 4 - n_dve:
                            # materialize BOTH operands contiguously (a
                            # broadcast operand in the DVE TT trips a
                            # 2.4x-slower perf-mode path, measured; two
                            # clean contiguous bf16 operands run true
                            # 2x_1P). Staging copies alternate between the
                            # otherwise-idle ScalarE and GPSIMD.
                            def stage(dst, src):
                                nc.scalar.copy(dst, src)

                            lrep = lpool.tile([P, F], BF, tag="lrep")
                            stage(
                                lrep[:, :].rearrange(
                                    "p (i h k) -> p i h k", h=16, k=16
                                ),
                                dv[:, tl, :, 0, :][:, :, None, :].broadcast_to(
                                    [P, 4, 16, 16]
                                ),
                            )
                            hrep = lpool.tile([P, F], BF, tag="hrep")
                            stage(
                                hrep[:, :].rearrange(
                                    "p (a k) -> p a k", k=16
                                ),
                                dhm_v[:, tl, :, None].broadcast_to(
                                    [P, 64, 16]
                                ),
                            )
                            nc.vector.tensor_mul(
                                o4[:, t4 * F : (t4 + 1) * F],
                                lrep[:, :],
                                hrep[:, :],
                            )
                        else:
                            l_b = dv[:, tl, :, 0, :][:, :, None, :].broadcast_to(
                                [P, 4, 16, 16]
                            )
                            h_b = dv[:, tl, :, 1, :][:, :, :, None].broadcast_to(
                                [P, 4, 16, 16]
                            )
                            nc.gpsimd.tensor_mul(o_v, l_b, h_b)
                    # early outputs ride the ACT HWDGE ring (SP is busy with
                    # the input stream); later ones use the by-then-idle SP
                    dma_eng = nc.scalar if ch == 0 else nc.sync
                    dma_eng.dma_start(out4_v[u_idx], o4[:, :])

            # ---- software pipeline: process(ch) runs while chunk ch+1's
            #      matmuls stream on the PE ----
            prev = None
            for ch in range(n_chunks):
                xnib = extract(ch)
                if prev is not None:
                    process(*prev)
                prev = (ch, xnib)
            flush_one_transpose_batch()
            flush_one_transpose_batch()
            process(*prev)

    nc.finalize()
    return nc


_NC_CACHE = {}
LAST_RESULT = None


def kernel(**inputs) -> np.ndarray:
    global LAST_RESULT
    a = np.ascontiguousarray(np.asarray(inputs["a"], dtype=np.float32)).reshape(
        B_FULL, F
    )
    b = np.ascontiguousarray(np.asarray(inputs["b"], dtype=np.float32)).reshape(
        B_FULL, F
    )
    # fp8 e4m3 staging: the one-hots are exactly 0.0/1.0 -> bytes 0x00/0x38.
    a8 = (a.view(np.uint16)[:, 1::2] != 0).astype(np.uint8) * np.uint8(0x38)
    b8 = (b.view(np.uint16)[:, 1::2] != 0).astype(np.uint8) * np.uint8(0x38)
    aT = np.ascontiguousarray(a8.reshape(N_CORES, ROWS, F).transpose(0, 2, 1))
    bT = np.ascontiguousarray(b8.reshape(N_CORES, ROWS, F).transpose(0, 2, 1))
    tab, ident, iota17 = _host_tables()

    if ROWS not in _NC_CACHE:
        _NC_CACHE[ROWS] = build_nc(ROWS)
    nc = _NC_CACHE[ROWS]

    in_maps = []
    for c in range(N_CORES):
        abT = np.concatenate([aT[c], bT[c]], axis=0).view(ml_dtypes.float8_e4m3)
        in_maps.append({"abT": abT, "tab": tab, "ident": ident, "iota17": iota17})
    res = run_bass_kernel_spmd(nc, in_maps, core_ids=list(range(N_CORES)))
    LAST_RESULT = res
    out16 = np.concatenate([r["out"] for r in res.results], axis=0)
    # bf16 -> fp32 exact expansion
    out32 = (out16.view(np.uint16).astype(np.uint32) << 16).view(np.float32)
    return out32.reshape(B_FULL, 4, 256)


# revision 50
# speedup vs baseline: 1.5299x; 1.0251x over previous
"""Trainium2 Bass kernel for nn_NeuralALU (batched byte-encoded 32-bit add).

The reference network is a chain of table-lookup matmuls + sharp softmaxes
(scale=100) over exactly-one-hot byte encodings. Because the inputs are exact
one-hots, the float pipeline collapses to a discrete algorithm (validated to
float-exactness against the jax reference in validate_math.py):

  z[2j]   = a_j%16 + b_j%16        (lo nibble sum of byte j, 0..30)
  z[2j+1] = a_j//16 + b_j//16      (hi nibble sum)
  carry c in {0, 0.5, 1}, init 0.5, chained over nibbles lo0,hi0,...,hi3:
      c' = max(g, min(p, c)),  g = [x>=16], p = [x>=15]
  add = [c==1]; y = x+add; U = y mod 16; Pf = [c==0.5]
  nibble dist = onehot(U)*(1-Pf/2) + onehot(U+1 mod 16)*(Pf/2)
  out byte row [256] = outer(hi_dist, lo_dist)

All staged values are exactly representable in the staging dtypes: the 0/1
one-hots and small-int nibble tables in fp8 e4m3, intermediate sums (<=31),
dist weights {0,.5,1} and outputs {0,.25,.5,1} in bf16 — so the kernel is
value-identical to the fp32 computation at a fraction of the HBM traffic.

Layout/engine strategy (pure data parallel over batch, 4096 rows/core):
 - Host pre-transposes each core's inputs to abT [2048 feat, 4096 rows] fp8
   so the (otherwise idle) tensor engine does the whole one-hot->nibble-sum
   extraction as matmuls against tiny [128, 2] nibble-value tables, packed
   4-wide into PE column groups {0,32,64,96} via tile_position (byte j's four
   K-chunks accumulate in its own column group -> no cross-group reduction).
 - ScalarE (also otherwise idle) drains PSUM; TensorE transposes z back to
   rows-on-partitions; the carry chain is ONE tensor_tensor_scan per chunk;
   dist build on DVE; the 256-wide outer products are split DVE/GPSIMD in
   4-row-tile quads; outputs stream back as bf16 in 1 MiB DMAs on the ACT
   HWDGE ring (inputs use SP's).
"""

import numpy as np
import ml_dtypes

import concourse.bass as bass
import concourse.bacc as bacc
import concourse.mybir as mybir
from concourse.tile import TileContext
from concourse.bass_utils import run_bass_kernel_spmd

N_CORES = 8
B_FULL = 32768
ROWS = B_FULL // N_CORES  # 4096 rows per core
F = 1024                  # 4 bytes x 256 one-hot
P = 128
NTC = 8                   # row-tiles per chunk (1024 rows = one input group)
HG = 512                  # matmul moving free dim (rows per half-group)

FP = mybir.dt.float32
BF = mybir.dt.bfloat16
F8 = mybir.dt.float8e4

# per (chunk, quad): how many of the quad's 4 row-tiles run their outer
# products on DVE (the rest go to GPSIMD); tuned from occupancy profiles
DVE_TILES = {(0, 0): 2, (0, 1): 2, (1, 0): 2, (1, 1): 2,
             (2, 0): 2, (2, 1): 2, (3, 0): 2, (3, 1): 1}
# broadcast-source staging copies run ONLY on ScalarE — GPSIMD executes
# them at ~4.4us/tile (5c/elem on stride-0 reads), measured


def _host_tables():
    # K-chunk c of abT covers features [128c, 128c+128) = half of one byte's
    # 256-wide one-hot block; its code k = 128*(c%2) + p. The per-chunk
    # [128, 2] nibble-value table (col 0: k%16, col 1: k//16) therefore only
    # depends on c's parity: tab[:, 2*(c%2) : 2*(c%2)+2].
    p = np.arange(128)
    tab = np.zeros((128, 4), np.float32)
    tab[:, 0] = p % 16
    tab[:, 1] = p // 16
    tab[:, 2] = p % 16
    tab[:, 3] = 8 + p // 16
    ident = np.eye(P, dtype=np.float32)
    # iota17[k] = (k+15) % 16 so one compare row yields onehot(U) at cols
    # 1..16 and onehot((U+1)%16) at cols 0..15
    i17 = ((np.arange(17) + 15) % 16).astype(np.float32)
    iota17 = np.broadcast_to(i17, (P, 17))
    return (
        np.ascontiguousarray(tab).astype(ml_dtypes.float8_e4m3),
        np.ascontiguousarray(ident),
        np.ascontiguousarray(iota17).astype(ml_dtypes.bfloat16),
    )


def build_nc(rows=ROWS):
    AL = mybir.AluOpType
    n_chunks = rows // (P * NTC)

    nc = bacc.Bacc()
    abT_d = nc.declare_dram_parameter("abT", [2 * F, rows], F8, isOutput=False)
    tab_d = nc.declare_dram_parameter("tab", [P, 4], F8, isOutput=False)
    ident_d = nc.declare_dram_parameter("ident", [P, P], FP, isOutput=False)
    iota_d = nc.declare_dram_parameter("iota17", [P, 17], BF, isOutput=False)
    out_d = nc.declare_dram_parameter("out", [rows, F], BF, isOutput=True)

    # input view: chunk c (of 16), partition p, row r  ->  abT[(c,p), r]
    ab_v = abT_d[:, :].rearrange("(c p) r -> p c r", p=P)
    # quad output view: quad u -> [p, t4, f] (4 row-tiles per DMA)
    out4_v = out_d[:, :].rearrange("(u t4 p) f -> u p t4 f", t4=4, p=P)

    with TileContext(nc) as tc:
        with (
            tc.tile_pool(name="consts", bufs=1) as cpool,
            tc.tile_pool(name="io", bufs=2) as iopool,
            tc.tile_pool(name="zsb", bufs=4) as zpool,
            tc.tile_pool(name="arrs", bufs=2) as apool,
            tc.tile_pool(name="dist", bufs=2) as dpool,
            tc.tile_pool(name="outp", bufs=3) as opool,
            tc.tile_pool(name="lrep", bufs=4) as lpool,
            tc.tile_pool(name="psz", bufs=2, space="PSUM") as psumz,
            tc.tile_pool(name="pst", bufs=5, space="PSUM") as psumt,
        ):
            tab_sb = cpool.tile([P, 4], F8, tag="tab")
            ident_sb = cpool.tile([P, P], FP, tag="ident")
            iota17 = cpool.tile([P, 17], BF, tag="iota17")
            nc.sync.dma_start(tab_sb[:, :], tab_d[:, :])
            nc.sync.dma_start(ident_sb[:, :], ident_d[:, :])
            nc.sync.dma_start(iota17[:, :], iota_d[:, :])

            # transposes lag the matmul stream by one half-group so the
            # PSUM-drain (zcopy) wait never stalls the PE's matmul queue
            pending_tr = []  # (zsb, xnib, chunk-local row-tile base)

            def flush_one_transpose_batch():
                if not pending_tr:
                    return
                zsb, xnib, tl0, ntiles = pending_tr.pop(0)
                for tt in range(ntiles):
                    tl = tl0 + tt
                    pst = psumt.tile([P, P], FP, tag="pst")
                    nc.tensor.transpose(
                        pst[:, :], zsb[:, tt * P : (tt + 1) * P], ident_sb[:, :]
                    )
                    # z columns live at partitions {32j, 32j+1} per byte j
                    # -> strided view recovers carry order lo0,hi0,...
                    pz_v = pst[:, :].rearrange("p (g q) -> p g q", g=4)[:, :, 0:2]
                    nc.scalar.copy(
                        xnib[:, tl * 8 : (tl + 1) * 8].rearrange(
                            "p (g q) -> p g q", g=4
                        ),
                        pz_v,
                    )

            def extract(ch):
                """Input DMA + matmul extraction; returns the xnib tile."""
                xnib = apool.tile([P, NTC * 8], BF, tag="xnib")
                grp = iopool.tile([P, 16 * 2 * HG], F8, tag="grp")
                grp_v = grp[:, :].rearrange("p (c r) -> p c r", c=16)
                r0 = ch * 2 * HG
                # chunk 0 split per half-group to shorten startup
                if ch == 0:
                    for hh in range(2):
                        nc.sync.dma_start(
                            grp_v[:, :, hh * HG : (hh + 1) * HG],
                            ab_v[:, :, r0 + hh * HG : r0 + (hh + 1) * HG],
                        )
                else:
                    nc.sync.dma_start(grp_v, ab_v[:, :, r0 : r0 + 2 * HG])

                # matmuls packed 4-wide into PE column groups {0,32,64,96}
                # (byte j's four K-chunks accumulate in its own column
                # group). N=512 per matmul (the ISA rejects N=1024:
                # s3d3_mm_num_elements).
                n_mm = 2
                W = (2 * HG) // n_mm
                for hh in range(n_mm):
                    psz = psumz.tile([P, W], FP, tag="psz")
                    for seq in range(4):
                        for j in range(4):
                            # chunks of byte j: a: 2j, 2j+1; b: 8+2j, 8+2j+1
                            c = (seq % 2) + 2 * j + 8 * (seq // 2)
                            nc.tensor.matmul(
                                psz[32 * j : 32 * j + 2, 0:W],
                                tab_sb[:, 2 * (c % 2) : 2 * (c % 2) + 2],
                                grp_v[:, c, hh * W : (hh + 1) * W],
                                start=(seq == 0),
                                stop=(seq == 3),
                                skip_group_check=True,
                                tile_position=(0, 32 * j),
                            )
                    zsb = zpool.tile([P, W], FP, tag="zsb")
                    nc.scalar.copy(zsb[:, 0:W], psz[:, 0:W])
                    flush_one_transpose_batch()
                    pending_tr.append((zsb, xnib, hh * (8 // n_mm), 8 // n_mm))
                return xnib

            def process(ch, xnib):
                # ---- carry scan over [reset-padded] nibble chains ----
                gp_g = apool.tile([P, NTC * 9], BF, tag="gpg")
                gp_p = apool.tile([P, NTC * 9], BF, tag="gpp")
                cbuf = apool.tile([P, NTC * 9 + 1], BF, tag="cbuf")
                gg_v = gp_g[:, :].rearrange("p (t n) -> p t n", n=9)
                pp_v = gp_p[:, :].rearrange("p (t n) -> p t n", n=9)
                xn_v = xnib[:, :].rearrange("p (t n) -> p t n", n=8)
                nc.vector.tensor_scalar(
                    out=gg_v[:, :, 0:8], in0=xn_v, scalar1=15.5, scalar2=None,
                    op0=AL.is_ge,
                )
                nc.vector.tensor_scalar(
                    out=pp_v[:, :, 0:8], in0=xn_v, scalar1=14.5, scalar2=None,
                    op0=AL.is_ge,
                )
                nc.vector.memset(gg_v[:, :, 8:9], 0.5)
                nc.vector.memset(pp_v[:, :, 8:9], 0.5)
                nc.vector.memset(cbuf[:, 0:1], 0.5)
                nc.vector.tensor_tensor_scan(
                    out=cbuf[:, 1 : NTC * 9 + 1],
                    data0=gp_p[:, :],
                    data1=gp_g[:, :],
                    initial=0.5,
                    op0=AL.min,
                    op1=AL.max,
                )
                c_pre = cbuf[:, 0 : NTC * 9].rearrange("p (t n) -> p t n", n=9)[
                    :, :, 0:8
                ]

                # ---- U / P-flag / dist weights ----
                add_a = apool.tile([P, NTC * 8], BF, tag="adda")
                pf = apool.tile([P, NTC * 8], BF, tag="pf")
                y_a = apool.tile([P, NTC * 8], BF, tag="ya")
                wrap = apool.tile([P, NTC * 8], BF, tag="wrap")
                u_a = apool.tile([P, NTC * 8], BF, tag="ua")
                w0 = apool.tile([P, NTC * 8], BF, tag="w0")
                w1 = apool.tile([P, NTC * 8], BF, tag="w1")
                av = add_a[:, :].rearrange("p (t n) -> p t n", n=8)
                pv = pf[:, :].rearrange("p (t n) -> p t n", n=8)
                nc.vector.tensor_scalar(
                    out=av, in0=c_pre, scalar1=0.75, scalar2=None, op0=AL.is_ge
                )
                nc.vector.tensor_scalar(
                    out=pv, in0=c_pre, scalar1=0.5, scalar2=None, op0=AL.is_equal
                )
                nc.vector.tensor_add(y_a[:, :], xnib[:, :], add_a[:, :])
                nc.vector.tensor_scalar(
                    out=wrap[:, :], in0=y_a[:, :], scalar1=15.5, scalar2=None,
                    op0=AL.is_ge,
                )
                nc.vector.scalar_tensor_tensor(
                    out=u_a[:, :], in0=wrap[:, :], scalar=-16.0, in1=y_a[:, :],
                    op0=AL.mult, op1=AL.add,
                )
                # w0/w1 are pure scale/bias of pf -> offload to idle ScalarE
                nc.scalar.mul(w1[:, :], pf[:, :], 0.5)
                nc.scalar.activation(
                    w0[:, :], pf[:, :], mybir.ActivationFunctionType.Identity,
                    bias=1.0, scale=-0.5,
                )

                # ---- dist build: one iota-compare then weighted combine
                #      (GPSIMD rejects compare ops — Pool engine ISA) ----
                TN = NTC * 8
                eqx = dpool.tile([P, TN * 17], BF, tag="eqx")
                dsub = dpool.tile([P, TN * 16], BF, tag="dsub")
                dtmp = dpool.tile([P, TN * 16], BF, tag="dtmp")
                # eqx[tn, k] = [U[tn] == (k+15)%16]: cols 1..16 = onehot(U),
                # cols 0..15 = onehot((U+1)%16)
                eq_v = eqx[:, :].rearrange("p (tn k) -> p tn k", k=17)
                u_b = u_a[:, :, None].broadcast_to([P, TN, 17])
                io_b = iota17[:, None, :].broadcast_to([P, TN, 17])
                nc.vector.tensor_tensor(eq_v, u_b, io_b, op=AL.is_equal)
                ds_v = dsub[:, :].rearrange("p (tn k) -> p tn k", k=16)
                dt_v = dtmp[:, :].rearrange("p (tn k) -> p tn k", k=16)
                w0_b = w0[:, :, None].broadcast_to([P, TN, 16])
                w1_b = w1[:, :, None].broadcast_to([P, TN, 16])
                nc.vector.tensor_mul(ds_v, eq_v[:, :, 1:17], w0_b)
                nc.vector.tensor_mul(dt_v, eq_v[:, :, 0:16], w1_b)
                nc.vector.tensor_add(dsub[:, :], dsub[:, :], dtmp[:, :])

                # merged contiguous copy of the hi dists -> the DVE outer's
                # h operand becomes a 3D AP (ScalarE pays the strided read)
                dhm = dpool.tile([P, NTC * 64], BF, tag="dhm")
                dv = dsub[:, :].rearrange(
                    "p (t i hf k) -> p t i hf k", i=4, hf=2, k=16
                )
                nc.scalar.copy(
                    dhm[:, :].rearrange("p (t i k) -> p t i k", i=4, k=16),
                    dv[:, :, :, 1, :],
                )
                dhm_v = dhm[:, :].rearrange("p (t ihk) -> p t ihk", t=NTC)

                # ---- outer products in 4-row-tile quads, split DVE/GPSIMD.
                #      DVE tiles get a ScalarE-staged contiguous l_rep so the
                #      TT's operands have <=2 free dims (the 3-free-dim
                #      broadcast AP runs ~2.4x slower on DVE) ----
                for q in range(NTC // 4):
                    u_idx = ch * (NTC // 4) + q
                    n_dve = DVE_TILES[(ch, q)]
                    o4 = opool.tile([P, 4 * F], BF, tag="o4")
                    for t4 in range(4):
                        o_v = o4[:, t4 * F : (t4 + 1) * F].rearrange(
                            "p (i h k) -> p i h k", h=16, k=16
                        )
                        tl = q * 4 + t4
                        if t4 >= 4 - n_dve:
                            # materialize BOTH operands contiguously (a
                            # broadcast operand in the DVE TT trips a
                            # 2.4x-slower perf-mode path, measured; two
                            # clean contiguous bf16 operands run true
                            # 2x_1P). Staging copies alternate between the
                            # otherwise-idle ScalarE and GPSIMD.
                            def stage(dst, src):
                                nc.scalar.copy(dst, src)

                            lrep = lpool.tile([P, F], BF, tag="lrep")
                            stage(
                                lrep[:, :].rearrange(
                                    "p (i h k) -> p i h k", h=16, k=16
                                ),
                                dv[:, tl, :, 0, :][:, :, None, :].broadcast_to(
                                    [P, 4, 16, 16]
                                ),
                            )
                            hrep = lpool.tile([P, F], BF, tag="hrep")
                            stage(
                                hrep[:, :].rearrange(
                                    "p (a k) -> p a k", k=16
                                ),
                                dhm_v[:, tl, :, None].broadcast_to(
                                    [P, 64, 16]
                                ),
                            )
                            nc.vector.tensor_mul(
                                o4[:, t4 * F : (t4 + 1) * F],
                                lrep[:, :],
                                hrep[:, :],
                            )
                        else:
                            l_b = dv[:, tl, :, 0, :][:, :, None, :].broadcast_to(
                                [P, 4, 16, 16]
                            )
                            h_b = dv[:, tl, :, 1, :][:, :, :, None].broadcast_to(
                                [P, 4, 16, 16]
                            )
                            nc.gpsimd.tensor_mul(o_v, l_b, h_b)
                    # early outputs ride the ACT HWDGE ring (SP is busy with
                    # the input stream); later ones use the by-then-idle SP
                    dma_eng = nc.scalar if ch == 0 else nc.sync
                    dma_eng.dma_start(out4_v[u_idx], o4[:, :])

            # ---- software pipeline: process(ch) runs while chunk ch+1's
            #      matmuls stream on the PE ----
            prev = None
            for ch in range(n_chunks):
                xnib = extract(ch)
                if prev is not None:
                    process(*prev)
                prev = (ch, xnib)
            flush_one_transpose_batch()
            flush_one_transpose_batch()
            process(*prev)

    nc.finalize()
    return nc


_NC_CACHE = {}
LAST_RESULT = None


def kernel(**inputs) -> np.ndarray:
    global LAST_RESULT
    a = np.ascontiguousarray(np.asarray(inputs["a"], dtype=np.float32)).reshape(
        B_FULL, F
    )
    b = np.ascontiguousarray(np.asarray(inputs["b"], dtype=np.float32)).reshape(
        B_FULL, F
    )
    # fp8 e4m3 staging: the one-hots are exactly 0.0/1.0 -> bytes 0x00/0x38.
    a8 = (a.view(np.uint16)[:, 1::2] != 0).astype(np.uint8) * np.uint8(0x38)
    b8 = (b.view(np.uint16)[:, 1::2] != 0).astype(np.uint8) * np.uint8(0x38)
    aT = np.ascontiguousarray(a8.reshape(N_CORES, ROWS, F).transpose(0, 2, 1))
    bT = np.ascontiguousarray(b8.reshape(N_CORES, ROWS, F).transpose(0, 2, 1))
    tab, ident, iota17 = _host_tables()

    if ROWS not in _NC_CACHE:
        _NC_CACHE[ROWS] = build_nc(ROWS)
    nc = _NC_CACHE[ROWS]

    in_maps = []
    for c in range(N_CORES):
        abT = np.concatenate([aT[c], bT[c]], axis=0).view(ml_dtypes.float8_e4m3)
        in_maps.append({"abT": abT, "tab": tab, "ident": ident, "iota17": iota17})
    res = run_bass_kernel_spmd(nc, in_maps, core_ids=list(range(N_CORES)))
    LAST_RESULT = res
    out16 = np.concatenate([r["out"] for r in res.results], axis=0)
    # bf16 -> fp32 exact expansion
    out32 = (out16.view(np.uint16).astype(np.uint32) << 16).view(np.float32)
    return out32.reshape(B_FULL, 4, 256)
